# revision 21
# baseline (speedup 1.0000x reference)
"""Trainium2 Bass kernel: 4-layer MLP (784-512-512-512-10) + log_softmax.

Data-parallel over 8 NeuronCores: batch 65536 is split into 8 shards of
8192 rows; the ~1M-param weights are replicated on every core.

Layout: activations live on-chip transposed ([features, batch]) so every
layer's matmul is `out[of, nb] += W_lT[if, of].T @ h[if, nb]` with the
feature chunks on partitions.  Layers 1-3 run in fp8e4 with DoubleRow
(fp32 PSUM accumulate); layer 4 runs in bf16 with the batch flipped onto
PSUM partitions so the softmax reduces along the free dim.  Batch is
processed in superchunks of 1024 rows; each output-feature chunk
accumulates two 512-column halves into one 2-bank PSUM pair so a single
1024-wide op applies bias+ReLU.

Key structure (vs the 170us first working version):
 - layer 1's K=784 is split as 8 chunks of 98 partitions: all four
   contraction passes are uniform DoubleRow, eliminating the 16-row
   remainder pass that cost a full 512-cycle stream per output chunk.
 - bias+ReLU PSUM drains alternate ScalarE (activation) / DVE
   (tensor_scalar add-bias-then-max-0) so neither engine gates PSUM
   recycling (all-ScalarE drains saturated it at ~13.4us/superchunk).
 - layer 4 lives in its own single PSUM bank ([128, 8*10] slices), so
   it never competes with the layer-1..3 PSUM ring, and its 8-matmul
   k-bursts are interleaved into layer 3's m-loop: each tiny-matmul
   burst is surrounded by dense DoubleRow streams, which keeps the PE
   HAM activity window busy (a contiguous low-activity stretch
   re-throttles the PE clock to 1.2GHz — measured as once-per-superchunk
   K=4/8 oscillation costing ~19us in one experiment).
 - the activation-table map is patched so Relu/Exp/Ln all resolve to
   the single natural_log_exp_and_others table: one ACT_TABLE_LOAD at
   start-up (preloaded via a dummy activation) instead of four 1.28us
   loads, two of which sat on the critical tail.
 - output is written partition-major ([128, 64, 10] per core, host
   un-transposes) so the final DMA uses KB-scale descriptors instead of
   40B ones (the old tail spent ~12us draining 40B descriptors).
 - weights + the first superchunk's x are split over the three DMA
   trigger queues (scalar/sync/gpsimd) in arrival-order-matched pieces;
   later x superchunks prefetch 8-deep on the sync queue.
 - PE warm-up: a few N=512 matmuls bridge the NEFF preamble to the
   first x/w chunk arrival so the HAM clock gate sees sustained
   activity (LDW-dominated warm-ups don't count as busy).
"""

from contextlib import ExitStack

import ml_dtypes
import numpy as np

import concourse.bass as bass  # noqa: F401  (registers AP machinery)
from concourse import bacc, hw_specs, mybir
from concourse.bass_utils import run_bass_kernel_spmd
from concourse.tile import TileContext

BF16 = mybir.dt.bfloat16
FP32 = mybir.dt.float32
FP8 = mybir.dt.float8e4

N_CORES = 8
B = 65536
D0, H, C = 784, 512, 10
BC = B // N_CORES            # 8192 rows per core
NB = 512                     # matmul moving free dim / PSUM bank width
HB = 2                       # batch halves sharing one PSUM group
SNB = NB * HB                # 1024-row superchunk
NCHUNK = BC // SNB           # 8 superchunks
KP = 98                      # layer-1 contraction chunk height (8*98=784)
K1 = D0 // KP                # 8 layer-1 chunks -> 4 DoubleRow passes
KH = H // 128                # 4 contraction chunks for hidden layers
NRG = BC // 128              # 64 row-groups of 128 rows per core
MG = SNB // 128              # 8 row-groups per superchunk

_CACHED_NC = None
_ACT_TABLES_PATCHED = False


def _patch_act_tables():
    """Make every activation resolve to the one table that holds
    relu+exp+ln together (natural_log_exp_and_others), so the kernel
    needs a single ACT_TABLE_LOAD instead of ping-ponging between the
    exp table and the ln table (1.28us per swap)."""
    global _ACT_TABLES_PATCHED
    if _ACT_TABLES_PATCHED:
        return
    _ACT_TABLES_PATCHED = True
    orig = hw_specs.get_activation_tables

    def patched(module_arch):
        tables = dict(orig(module_arch))
        keep = "natural_log_exp_and_others"
        if keep in tables:
            tables = {
                name: (funcs if name == keep else set())
                for name, funcs in tables.items()
            }
        return tables

    hw_specs.get_activation_tables = patched
    bacc.get_activation_tables = patched


def build_nc():
    _patch_act_tables()
    nc = bacc.Bacc(
        "TRN2",
        target_bir_lowering=False,
        debug=False,
        enable_asserts=False,
        num_devices=N_CORES,
    )
    # host-packed layouts: per partition, all chunks contiguous (multi-KB
    # DMA runs — 512B-line weight loads measured only ~60GB/s/queue,
    # descriptor-generation-bound, landing w3/w4 at 15-22us and stalling
    # the first superchunk).
    xt_d = nc.declare_dram_parameter("xh", [KP, NCHUNK * K1 * SNB], FP8, isOutput=False)
    w1_d = nc.declare_dram_parameter("w1h", [KP, K1 * H], FP8, isOutput=False)
    w2_d = nc.declare_dram_parameter("w2h", [128, KH * H], FP8, isOutput=False)
    w3_d = nc.declare_dram_parameter("w3h", [128, KH * H], FP8, isOutput=False)
    w4_d = nc.declare_dram_parameter("w4r", [128, KH, C], BF16, isOutput=False)
    br_d = nc.declare_dram_parameter("brec", [128, 3 * KH + C], FP32, isOutput=False)
    # partition-major output: [p, rg, c] holds batch row rg*128 + p
    out_d = nc.declare_dram_parameter("out", [128, NRG, C], FP32, isOutput=True)

    expf = mybir.ActivationFunctionType.Exp
    reluf = mybir.ActivationFunctionType.Relu
    lnf = mybir.ActivationFunctionType.Ln
    add_op = mybir.AluOpType.add
    max_op = mybir.AluOpType.max
    sub_op = mybir.AluOpType.subtract
    drow = mybir.MatmulPerfMode.DoubleRow

    with TileContext(nc) as tc, ExitStack() as ctx:
        consts = ctx.enter_context(tc.tile_pool(name="consts", bufs=1))
        xpool = ctx.enter_context(tc.tile_pool(name="xp", bufs=8))
        hpool = ctx.enter_context(tc.tile_pool(name="hp", bufs=3))
        spool = ctx.enter_context(tc.tile_pool(name="sp", bufs=4))
        pbig = ctx.enter_context(tc.tile_pool(name="pbig", bufs=3, space="PSUM"))
        psml = ctx.enter_context(tc.tile_pool(name="psml", bufs=1, space="PSUM"))

        warm = consts.tile([128, NB], FP8, tag="warm")
        nc.vector.memset(warm[:], 1.0)

        # ---- resident weights/biases over the three DMA trigger queues,
        # ordered to match first-use times ----
        # scalar queue: w1 (fine-grained head), table preload, w4.
        w1 = consts.tile([KP, K1, H], FP8, tag="w1")
        nc.scalar.dma_start(
            w1[:, 0:2, :],
            w1_d[:, 0 : 2 * H].rearrange("p (k n) -> p k n", n=H),
        )
        nc.scalar.dma_start(
            w1[:, 2:8, :],
            w1_d[:, 2 * H : 8 * H].rearrange("p (k n) -> p k n", n=H),
        )
        scratch = consts.tile([128, 4], FP32, tag="scratch")
        nc.scalar.activation(scratch[:, 0:1], warm[:, 0:1], reluf)
        w4 = consts.tile([128, KH, C], BF16, tag="w4")
        nc.scalar.dma_start(w4[:], w4_d[:])
        # gpsimd queue: biases, w2, w3.
        brec = consts.tile([128, 3 * KH + C], FP32, tag="brec")
        nc.gpsimd.dma_start(brec[:], br_d[:])
        w2 = consts.tile([128, KH, H], FP8, tag="w2")
        nc.gpsimd.dma_start(w2[:], w2_d.rearrange("p (k n) -> p k n", n=H))
        w3 = consts.tile([128, KH, H], FP8, tag="w3")
        nc.gpsimd.dma_start(w3[:], w3_d.rearrange("p (k n) -> p k n", n=H))
        b4s = brec[:, 3 * KH : 3 * KH + C]
        # sync queue: x0 in two pieces, then x1..x7, 8-deep prefetch
        # (8KB contiguous per partition per superchunk).
        SCW = K1 * SNB
        xt0 = xpool.tile([KP, K1, SNB], FP8, tag="xt", name="xt_0")
        nc.sync.dma_start(
            xt0[:, 0:2, :],
            xt_d[:, 0 : 2 * SNB].rearrange("p (k n) -> p k n", n=SNB),
        )
        nc.sync.dma_start(
            xt0[:, 2:8, :],
            xt_d[:, 2 * SNB : SCW].rearrange("p (k n) -> p k n", n=SNB),
        )
        xts = [xt0]
        for sc in range(1, NCHUNK):
            xt = xpool.tile([KP, K1, SNB], FP8, tag="xt", name=f"xt_{sc}")
            nc.sync.dma_start(
                xt[:],
                xt_d[:, sc * SCW : (sc + 1) * SCW].rearrange(
                    "p (k n) -> p k n", n=SNB
                ),
            )
            xts.append(xt)

        # ---- PE warm-up: N=512 streams keep the HAM activity counter
        # fed from the end of the NEFF preamble to first-data. ----
        psw = pbig.tile([128, HB, NB], FP32, tag="ps", name="ps_warm")
        NWARM = 5
        for i in range(NWARM):
            nc.tensor.matmul(
                psw[:, i % 2, :], lhsT=warm[:, 0:128], rhs=warm[:],
                start=(i < 2), stop=(i >= NWARM - 2),
            )

        # Persistent softmax state for all 64 row-groups.
        logits_all = consts.tile([128, NRG, C], FP32, tag="logits_all")
        esum_all = consts.tile([128, NRG], FP32, tag="esum_all")
        lns_all = consts.tile([128, NRG], FP32, tag="lns_all")
        obuf = consts.tile([128, NRG, C], FP32, tag="obuf")

        def softmax_epilogue(rg0, rg1):
            # out = logits - ln(sum(exp(logits))) for row-groups [rg0, rg1)
            n = rg1 - rg0
            nc.scalar.activation(lns_all[:, rg0:rg1], esum_all[:, rg0:rg1], lnf)
            nc.vector.tensor_tensor(
                obuf[:, rg0:rg1, :], logits_all[:, rg0:rg1, :],
                lns_all[:, rg0:rg1, None].to_broadcast((128, n, C)), sub_op,
            )
            nc.sync.dma_start(out_d[:, rg0:rg1, :], obuf[:, rg0:rg1, :])

        def drain(engine_is_dve, dst, ps, bias_ap):
            # PSUM -> SBUF bias+ReLU, alternating engines so neither
            # gates PE PSUM recycling.
            if engine_is_dve:
                nc.vector.tensor_scalar(dst, ps, bias_ap, 0.0, add_op, max_op)
            else:
                nc.scalar.activation(dst, ps, reluf, bias=bias_ap)

        def l4_burst(ps4, h3, k):
            # Layer 4 [512 -> 10] contribution of h3 chunk k: 8 tiny bf16
            # matmuls (one per 128-row group), each an independent
            # single-shot write into region k of the psml bank.  No
            # accumulating read-modify-write chains across different
            # addresses of one bank (that pattern measured slightly
            # wrong results on hardware); the four regions are summed on
            # DVE instead.
            for hb in range(HB):
                for mm in range(NB // 128):
                    r = hb * (NB // 128) + mm
                    nc.tensor.matmul(
                        ps4[:, (k * MG + r) * C : (k * MG + r + 1) * C],
                        lhsT=h3[k][:, hb, mm * 128 : (mm + 1) * 128],
                        rhs=w4[:, k, :],
                        start=True, stop=True,
                    )

        def ps4r(ps4, k):
            return ps4[:, k * MG * C : (k + 1) * MG * C].rearrange(
                "p (r c) -> p r c", c=C
            )

        def l4_finish(st):
            # DVE region-sum for the previous superchunk's layer 4 (all
            # four region writes happened before any of these reads —
            # Tile tracks PSUM deps at tile granularity, so interleaving
            # writes and reads of ps4 serializes them), then exp/sum-exp.
            ps4, tmp, h3, rg0 = st
            tmp3 = tmp[:].rearrange("p (r c) -> p r c", c=C)
            nc.vector.tensor_tensor(
                tmp3, ps4r(ps4, 0),
                b4s[:, None, :].to_broadcast((128, MG, C)), add_op,
            )
            nc.vector.tensor_tensor(tmp3, tmp3, ps4r(ps4, 1), add_op)
            nc.vector.tensor_tensor(tmp3, tmp3, ps4r(ps4, 2), add_op)
            lg = logits_all[:, rg0 : rg0 + MG, :]
            nc.vector.tensor_tensor(lg, tmp3, ps4r(ps4, 3), add_op)
            etile = spool.tile([128, MG, C], FP32, tag="etile")
            nc.scalar.activation(etile[:], lg, expf)
            nc.vector.tensor_reduce(
                esum_all[:, rg0 : rg0 + MG], etile[:],
                axis=mybir.AxisListType.X, op=add_op,
            )

        prev = None
        for sc in range(NCHUNK):
            xt = xts[sc]
            rg0 = sc * MG

            # ---- Layer 1 [784 -> 512]: 4 uniform fp8 DoubleRow passes ----
            h1p = [
                hpool.tile([128, 2, HB, NB], FP8, tag=f"h1p_{j}", name=f"h1p_{j}")
                for j in range(KH // 2)
            ]
            for m in range(KH):
                ps = pbig.tile([128, HB, NB], FP32, tag="ps")
                ms = slice(m * 128, (m + 1) * 128)
                for kp in range(K1 // 2):
                    for hb in range(HB):
                        nc.tensor.matmul(
                            ps[:, hb, :], lhsT=w1[:, 2 * kp : 2 * kp + 2, ms],
                            rhs=xt[:, 2 * kp : 2 * kp + 2, hb * NB : (hb + 1) * NB],
                            start=(kp == 0), stop=(kp == K1 // 2 - 1),
                            perf_mode=drow,
                        )
                drain(
                    m % 2 == 1,
                    h1p[m // 2][:, m % 2, :, :], ps[:], brec[:, m : m + 1],
                )
                # previous superchunk's layer-4 tail rides between this
                # superchunk's dense layer-1 streams.
                if prev is not None and m == 0:
                    l4_burst(prev[0], prev[2], 3)
                if prev is not None and m == 1:
                    l4_finish(prev)
                    prev = None
                    if sc == NCHUNK - 1:
                        # most of the softmax epilogue hides under the
                        # last superchunk's compute.
                        softmax_epilogue(0, (NCHUNK - 1) * MG)

            # ---- Layer 2 [512 -> 512]: fp8 DoubleRow over chunk pairs ----
            h2p = [
                hpool.tile([128, 2, HB, NB], FP8, tag=f"h2p_{j}", name=f"h2p_{j}")
                for j in range(KH // 2)
            ]
            for m in range(KH):
                ps = pbig.tile([128, HB, NB], FP32, tag="ps")
                ms = slice(m * 128, (m + 1) * 128)
                for j in range(KH // 2):
                    for hb in range(HB):
                        nc.tensor.matmul(
                            ps[:, hb, :], lhsT=w2[:, 2 * j : 2 * j + 2, ms],
                            rhs=h1p[j][:, :, hb, :],
                            start=(j == 0), stop=(j == KH // 2 - 1),
                            perf_mode=drow,
                        )
                drain(
                    m % 2 == 1,
                    h2p[m // 2][:, m % 2, :, :], ps[:], brec[:, KH + m : KH + m + 1],
                )

            # ---- Layer 3 [512 -> 512] with layer 4's k-bursts woven in:
            # each burst of 8 tiny matmuls only needs h3[k], which drained
            # during the following m-chunk's DoubleRow streams. ----
            h3 = [
                hpool.tile([128, HB, NB], BF16, tag=f"h3_{m}", name=f"h3_{m}")
                for m in range(KH)
            ]
            ps4 = psml.tile([128, KH * MG * C], FP32, tag="ps4")
            tmp = spool.tile([128, MG * C], FP32, tag="l4tmp")
            for m in range(KH):
                ps = pbig.tile([128, HB, NB], FP32, tag="ps")
                ms = slice(m * 128, (m + 1) * 128)
                for j in range(KH // 2):
                    for hb in range(HB):
                        nc.tensor.matmul(
                            ps[:, hb, :], lhsT=w3[:, 2 * j : 2 * j + 2, ms],
                            rhs=h2p[j][:, :, hb, :],
                            start=(j == 0), stop=(j == KH // 2 - 1),
                            perf_mode=drow,
                        )
                drain(
                    m % 2 == 1,
                    h3[m][:], ps[:], brec[:, 2 * KH + m : 2 * KH + m + 1],
                )
                if m == KH - 2:
                    l4_burst(ps4, h3, 0)
                elif m == KH - 1:
                    l4_burst(ps4, h3, 1)
            l4_burst(ps4, h3, 2)
            prev = (ps4, tmp, h3, rg0)

        l4_burst(prev[0], prev[2], 3)
        l4_finish(prev)
        softmax_epilogue((NCHUNK - 1) * MG, NRG)

    nc.compile()
    return nc


def _get_nc():
    global _CACHED_NC
    if _CACHED_NC is None:
        _CACHED_NC = build_nc()
    return _CACHED_NC


def make_in_maps(x, W1, b1, W2, b2, W3, b3, W4, b4):
    bf16 = ml_dtypes.bfloat16
    fp8 = ml_dtypes.float8_e4m3
    xq = np.asarray(x).astype(fp8)
    w4r = (
        np.asarray(W4).T.astype(bf16)              # [512, 10]
        .reshape(KH, 128, C).transpose(1, 0, 2)    # [128, 4, 10]
    )
    brec = np.hstack(
        [
            np.asarray(b1).astype(np.float32).reshape(KH, 128).T,
            np.asarray(b2).astype(np.float32).reshape(KH, 128).T,
            np.asarray(b3).astype(np.float32).reshape(KH, 128).T,
            np.tile(np.asarray(b4).astype(np.float32)[None, :], (128, 1)),
        ]
    )
    def pack_k(wT, kp):
        # [K, N] -> [kp, K//kp * N]: row k*kp+p lands at [p, k*N : (k+1)*N]
        k, n = wT.shape
        return np.ascontiguousarray(
            wT.reshape(k // kp, kp, n).transpose(1, 0, 2).reshape(kp, -1)
        )

    common = {
        "w1h": pack_k(np.asarray(W1).T.astype(fp8), KP),
        "w2h": pack_k(np.asarray(W2).T.astype(fp8), 128),
        "w3h": pack_k(np.asarray(W3).T.astype(fp8), 128),
        "w4r": np.ascontiguousarray(w4r),
        "brec": np.ascontiguousarray(brec),
    }
    in_maps = []
    for i in range(N_CORES):
        shard = xq[i * BC : (i + 1) * BC].T        # [784, 8192]
        # [98, sc, k, 1024]: per partition, each superchunk's 8 k-chunks
        # are one contiguous 8KB DMA run.
        xh = np.ascontiguousarray(
            shard.reshape(K1, KP, NCHUNK, SNB)
            .transpose(1, 2, 0, 3)
            .reshape(KP, -1)
        )
        in_maps.append({"xh": xh, **common})
    return in_maps


def gather_out(res):
    # out is [128, 64, 10] partition-major per core: row rg*128 + p
    # lives at [p, rg, :].
    return np.concatenate(
        [
            np.asarray(res.results[i]["out"])
            .transpose(1, 0, 2)
            .reshape(BC, C)
            for i in range(N_CORES)
        ],
        axis=0,
    ).astype(np.float32)


def kernel(x, W1, b1, W2, b2, W3, b3, W4, b4):
    in_maps = make_in_maps(x, W1, b1, W2, b2, W3, b3, W4, b4)
    nc = _get_nc()
    res = run_bass_kernel_spmd(nc, in_maps, list(range(N_CORES)))
    return gather_out(res)
